# revision 1
# baseline (speedup 1.0000x reference)
"""Child-Sum TreeLSTM over a complete 8-ary tree (depth 6, 299593 nodes) on
8 Trainium2 NeuronCores.

Sharding: the 8 independent subtrees under the root go one-per-core; each core
runs the full bottom-up sweep for its subtree (37449 nodes) and returns the
(c, h) state of its subtree root (= one of the root's children). The final
root step (8 children, [1,128]) runs on the host in fp32 after the gather.

Device layout: feature-major ([128 features on partitions, nodes on free dim]).
x is pre-transposed/cast to fp16 on the host so DMA loads need no transpose.
Matmuls run in fp16 (fp32 PSUM accumulation), gates in fp16 via the ACT engine
(sigmoid/tanh with per-partition bias = the folded b_* vectors), child-sum
reductions as fp16 tree-adds on the vector engine. The per-child forget-gate
pre-activation accumulates W_fh@h_child and W_fx@x_parent into the same PSUM
via a second matmul whose rhs is the parent's x broadcast 8x (0-stride AP).
Small levels (<=512 nodes) use a latency-optimized path: biases added via K=1
matmuls so sigmoid(i) and sigmoid(o) merge into one ACT op, and child-sums as
single tensor_reduce ops.
"""

import os

import numpy as np

import concourse.bass as bass
import concourse.tile as tile
from concourse import bacc, mybir
from concourse.bass_utils import run_bass_kernel_spmd

F16 = mybir.dt.float16
F32 = mybir.dt.float32
SIG = mybir.ActivationFunctionType.Sigmoid
TANH = mybir.ActivationFunctionType.Tanh

BRANCH = 8
DEPTH = 6
MEM = 128
IN_DIM = 128
N_NODES = (BRANCH ** (DEPTH + 1) - 1) // (BRANCH - 1)  # 299593

# Per-subtree (local) levels L1..L6: sizes 8^(L-1), offsets into the per-core
# x tensor (levels concatenated in order L1..L6).
LVL_SIZES = [BRANCH**i for i in range(DEPTH)]  # [1, 8, 64, 512, 4096, 32768]
LVL_OFF = [sum(LVL_SIZES[:i]) for i in range(DEPTH)]  # [0,1,9,73,585,4681]
SUB_N = sum(LVL_SIZES)  # 37449

LEAF_OFF = LVL_OFF[5]  # 4681
L5_OFF = LVL_OFF[4]  # 585
L4_OFF = LVL_OFF[3]  # 73
N_CHUNKS = 8  # leaf/L5 fused chunks: 512 L5-parents (4096 leaves) each

LAST_RESULTS = None  # stash for test harness introspection


def _tree_reduce8(nc, pool, src3, m, dst, tag):
    """dst[128, m] = sum over last axis of src3 [128, m, 8] (fp16 tree adds)."""
    t1 = pool.tile([128, m * 4], F16, tag=tag + "_t1")
    t1v = t1[:].rearrange("p (m f) -> p m f", f=4)
    nc.vector.tensor_add(t1v, src3[:, :, 0:4], src3[:, :, 4:8])
    t2 = pool.tile([128, m * 2], F16, tag=tag + "_t2")
    t2v = t2[:].rearrange("p (m f) -> p m f", f=2)
    nc.vector.tensor_add(t2v, t1v[:, :, 0:2], t1v[:, :, 2:4])
    dstv = dst.rearrange("p (m f) -> p m f", f=1)
    nc.vector.tensor_add(dstv, t2v[:, :, 0:1], t2v[:, :, 1:2])


def _gate_group(nc, psum, gg, W, m, c_ch, h_ch, x_par, hs_dst, fc_dst, small=False):
    """Forget gates + child sums for m parents (8m children).

    c_ch/h_ch: [128, 8m] fp16 child states; x_par: [128, m] fp16 parent x.
    Writes hs_dst/fc_dst [128, m] (h child-sum, sum of f*c).
    small=True: use single tensor_reduce ops (fp32 dsts) for low latency.
    """
    cols = 8 * m
    pf = psum.tile([128, cols], F32, tag="pg")
    # weight-major order: all wfh matmuls, then all wfx, so LDWEIGHTS is not
    # re-issued per matmul (keeps PE fill rate ahead of the ACT reader).
    for s in range(0, cols, 512):
        e = min(cols, s + 512)
        nc.tensor.matmul(pf[:, s:e], W["wfh"][:], h_ch[:, s:e], start=True, stop=False)
    for s in range(0, cols, 512):
        e = min(cols, s + 512)
        xb = (
            x_par[:, s // 8 : e // 8]
            .rearrange("p (m o) -> p m o", o=1)
            .broadcast_to([128, (e - s) // 8, 8])
        )
        nc.tensor.matmul(pf[:, s:e], W["wfx"][:], xb, start=False, stop=True)
    f = gg.tile([128, cols], F16, tag="f")
    nc.scalar.activation(f[:], pf[:], SIG, bias=W["bf"][:])
    prod = gg.tile([128, cols], F16, tag="prod")
    nc.vector.tensor_mul(prod[:], f[:], c_ch)
    prod3 = prod[:].rearrange("p (m f) -> p m f", f=8)
    hch3 = h_ch.rearrange("p (m f) -> p m f", f=8)
    if small:
        with nc.allow_low_precision("8-element fp16 child-sum"):
            nc.vector.tensor_reduce(
                fc_dst, prod3, axis=mybir.AxisListType.X, op=mybir.AluOpType.add
            )
            nc.vector.tensor_reduce(
                hs_dst, hch3, axis=mybir.AxisListType.X, op=mybir.AluOpType.add
            )
    else:
        _tree_reduce8(nc, gg, prod3, m, fc_dst, "fc")
        _tree_reduce8(nc, gg, hch3, m, hs_dst, "hs")


def _level_top(nc, psum, gp, W, n, x_l, hs, fc, c_dst, h_dst):
    """iou gates + cell update for n nodes (big-level path, per-gate ACTs).

    x_l [128, n] fp16; hs/fc [128, n] fp16 (None for leaves);
    writes c_dst/h_dst [128, n] fp16.
    """

    def iou_psum(gate):
        p = psum.tile([128, n], F32, tag="pg")
        w = W["wioux"][:, gate * 128 : (gate + 1) * 128]
        for s in range(0, n, 512):
            e = min(n, s + 512)
            nc.tensor.matmul(p[:, s:e], w, x_l[:, s:e], start=True, stop=hs is None)
        if hs is not None:
            wh = W["wiouh"][:, gate * 128 : (gate + 1) * 128]
            for s in range(0, n, 512):
                e = min(n, s + 512)
                nc.tensor.matmul(p[:, s:e], wh, hs[:, s:e], start=False, stop=True)
        return p

    bi = W["biou"][:, 0:1]
    bo = W["biou"][:, 1:2]
    bu = W["biou"][:, 2:3]

    pi = iou_psum(0)
    pu = iou_psum(2)
    si = gp.tile([128, n], F16, tag="si")
    nc.scalar.activation(si[:], pi[:], SIG, bias=bi)
    tu = gp.tile([128, n], F16, tag="tu")
    nc.scalar.activation(tu[:], pu[:], TANH, bias=bu)
    po = iou_psum(1)
    if fc is None:
        nc.vector.tensor_mul(c_dst, si[:], tu[:])
    else:
        ct = gp.tile([128, n], F16, tag="ct")
        nc.vector.tensor_mul(ct[:], si[:], tu[:])
        nc.vector.tensor_add(c_dst, ct[:], fc)
    so = gp.tile([128, n], F16, tag="so")
    nc.scalar.activation(so[:], po[:], SIG, bias=bo)
    tct = gp.tile([128, n], F16, tag="tct")
    nc.scalar.activation(tct[:], c_dst, TANH)
    nc.vector.tensor_mul(h_dst, so[:], tct[:])


def _level_top_small(nc, psum, gp, W, n, x_l, hs, fc, c_dst, h_dst, out_dt=F16):
    """iou gates + cell update, latency-optimized for small n (<=512).

    Biases are added in PSUM via K=1 matmuls (ones-rhs) so sigmoid(i) and
    sigmoid(o) can run as one ACT op over [128, 2n].
    """
    p = psum.tile([128, 3 * n], F32, tag="pg")
    ones = W["ones"]
    for gate in range(3):
        sl = slice(gate * n, (gate + 1) * n)
        w = W["wioux"][:, gate * 128 : (gate + 1) * 128]
        nc.tensor.matmul(p[:, sl], w, x_l, start=True, stop=False)
        wh = W["wiouh"][:, gate * 128 : (gate + 1) * 128]
        nc.tensor.matmul(p[:, sl], wh, hs, start=False, stop=False)
        br = W["biourow"][:, gate * 128 : (gate + 1) * 128]
        nc.tensor.matmul(p[:, sl], br, ones[:, 0:n], start=False, stop=True)

    sio = gp.tile([128, 2 * n], F16, tag="sio")
    nc.scalar.activation(sio[:], p[:, 0 : 2 * n], SIG)
    tu = gp.tile([128, n], F16, tag="tu")
    nc.scalar.activation(tu[:], p[:, 2 * n : 3 * n], TANH)
    ct = gp.tile([128, n], F16, tag="ct")
    nc.vector.tensor_mul(ct[:], sio[:, 0:n], tu[:])
    nc.vector.tensor_add(c_dst, ct[:], fc)
    tct = gp.tile([128, n], out_dt, tag="tct")
    nc.scalar.activation(tct[:], c_dst, TANH)
    nc.vector.tensor_mul(h_dst, sio[:, n : 2 * n], tct[:])


def _build_subtree_kernel():
    nc = bacc.Bacc("TRN2", target_bir_lowering=False, debug=False, num_devices=8)

    xs = nc.dram_tensor("xs", [128, SUB_N], F16, kind="ExternalInput").ap()
    wioux_d = nc.dram_tensor("wioux", [128, 384], F16, kind="ExternalInput").ap()
    wiouh_d = nc.dram_tensor("wiouh", [128, 384], F16, kind="ExternalInput").ap()
    wfx_d = nc.dram_tensor("wfx", [128, 128], F16, kind="ExternalInput").ap()
    wfh_d = nc.dram_tensor("wfh", [128, 128], F16, kind="ExternalInput").ap()
    biou_d = nc.dram_tensor("biou", [128, 3], F32, kind="ExternalInput").ap()
    biourow_d = nc.dram_tensor("biourow", [1, 384], F16, kind="ExternalInput").ap()
    bf_d = nc.dram_tensor("bf", [128, 1], F32, kind="ExternalInput").ap()
    out_d = nc.dram_tensor("out", [128, 2], F32, kind="ExternalOutput").ap()

    with tile.TileContext(nc) as tc:
        with (
            tc.tile_pool(name="const", bufs=1) as cp,
            tc.tile_pool(name="xlo", bufs=1) as xlo_p,
            tc.tile_pool(name="x6", bufs=3) as x6_p,
            tc.tile_pool(name="leafst", bufs=2) as lf_p,
            tc.tile_pool(name="state", bufs=1) as st,
            tc.tile_pool(name="gates", bufs=2) as gp,
            tc.tile_pool(name="gg", bufs=2) as gg,
            tc.tile_pool(name="psum", bufs=2, space="PSUM") as psum,
        ):
            # --- constants: weights needed by the first leaf matmuls go on the
            # fast HWDGE queue first; everything else on the gpsimd queue so it
            # doesn't delay the first x chunk.
            W = {}

            def load_const(name, dram, shape, dt, engine):
                t = cp.tile(shape, dt, tag=name)
                engine.dma_start(t[:], dram)
                W[name] = t

            # first leaf chunk half, ASAP on the sync queue
            x6_tiles = {}
            x6_tiles[(0, 0)] = x6_p.tile([128, 2048], F16, tag="x6", name="x6_0_0")
            nc.sync.dma_start(
                x6_tiles[(0, 0)][:], xs[:, LEAF_OFF : LEAF_OFF + 2048]
            )
            load_const("wioux", wioux_d, [128, 384], F16, nc.sync)
            load_const("biou", biou_d, [128, 3], F32, nc.sync)

            load_const("wiouh", wiouh_d, [128, 384], F16, nc.gpsimd)
            load_const("wfx", wfx_d, [128, 128], F16, nc.gpsimd)
            load_const("wfh", wfh_d, [128, 128], F16, nc.gpsimd)
            load_const("biourow", biourow_d, [1, 384], F16, nc.gpsimd)
            load_const("bf", bf_d, [128, 1], F32, nc.gpsimd)
            ones = cp.tile([1, 512], F16, tag="ones")
            nc.vector.memset(ones[:], 1.0)
            W["ones"] = ones

            x6_tiles[(0, 1)] = x6_p.tile([128, 2048], F16, tag="x6", name="x6_0_1")
            nc.sync.dma_start(
                x6_tiles[(0, 1)][:], xs[:, LEAF_OFF + 2048 : LEAF_OFF + 4096]
            )

            # x for levels L1..L5 (cols 0..4681), persistent; on gpsimd queue
            x15 = xlo_p.tile([128, LEAF_OFF], F16)
            nc.gpsimd.dma_start(x15[:], xs[:, 0:LEAF_OFF])

            # persistent state/partials
            hs5 = st.tile([128, 4096], F16, tag="hs5")
            fc5 = st.tile([128, 4096], F16, tag="fc5")
            c5 = st.tile([128, 4096], F16, tag="c5")
            h5 = st.tile([128, 4096], F16, tag="h5")
            hs4 = st.tile([128, 512], F16, tag="hs4")
            fc4 = st.tile([128, 512], F16, tag="fc4")
            c4 = st.tile([128, 512], F16, tag="c4")
            h4 = st.tile([128, 512], F16, tag="h4")
            hs3 = st.tile([128, 64], F16, tag="hs3")
            fc3 = st.tile([128, 64], F16, tag="fc3")
            c3 = st.tile([128, 64], F16, tag="c3")
            h3 = st.tile([128, 64], F16, tag="h3")
            hs2 = st.tile([128, 8], F16, tag="hs2")
            fc2 = st.tile([128, 8], F16, tag="fc2")
            c2 = st.tile([128, 8], F16, tag="c2")
            h2 = st.tile([128, 8], F16, tag="h2")
            hs1 = st.tile([128, 1], F16, tag="hs1")
            fc1 = st.tile([128, 1], F16, tag="fc1")
            out_ch = st.tile([128, 2], F32, tag="out_ch")

            def _iou_psum_1024(g, gate):
                sl = slice(g * 1024, (g + 1) * 1024)
                x_l = x15[:, L5_OFF + sl.start : L5_OFF + sl.stop]
                p = psum.tile([128, 1024], F32, tag="pg", name=f"p5_{g}_{gate}")
                w = W["wioux"][:, gate * 128 : (gate + 1) * 128]
                for s in range(0, 1024, 512):
                    nc.tensor.matmul(
                        p[:, s : s + 512], w, x_l[:, s : s + 512],
                        start=True, stop=False,
                    )
                wh = W["wiouh"][:, gate * 128 : (gate + 1) * 128]
                for s in range(0, 1024, 512):
                    nc.tensor.matmul(
                        p[:, s : s + 512], wh, hs5[:, sl][:, s : s + 512],
                        start=False, stop=True,
                    )
                return p

            def l5top_p1(g):
                # sigmoid(i), tanh(u), c = si*tu + fc for the g-th quarter
                sl = slice(g * 1024, (g + 1) * 1024)
                pi = _iou_psum_1024(g, 0)
                pu = _iou_psum_1024(g, 2)
                si = gp.tile([128, 1024], F16, tag="si", name=f"si5_{g}")
                nc.scalar.activation(si[:], pi[:], SIG, bias=W["biou"][:, 0:1])
                tu = gp.tile([128, 1024], F16, tag="tu", name=f"tu5_{g}")
                nc.scalar.activation(tu[:], pu[:], TANH, bias=W["biou"][:, 2:3])
                ct = gp.tile([128, 1024], F16, tag="ct", name=f"ct5_{g}")
                nc.vector.tensor_mul(ct[:], si[:], tu[:])
                nc.vector.tensor_add(c5[:, sl], ct[:], fc5[:, sl])

            def l5top_p2(g):
                # sigmoid(o), tanh(c), h = so*tc for the g-th quarter
                sl = slice(g * 1024, (g + 1) * 1024)
                po = _iou_psum_1024(g, 1)
                so = gp.tile([128, 1024], F16, tag="so", name=f"so5_{g}")
                nc.scalar.activation(so[:], po[:], SIG, bias=W["biou"][:, 1:2])
                tct = gp.tile([128, 1024], F16, tag="tct", name=f"tct5_{g}")
                nc.scalar.activation(tct[:], c5[:, sl], TANH)
                nc.vector.tensor_mul(h5[:, sl], so[:], tct[:])

            def l4gate(g):
                sl = slice(g * 1024, (g + 1) * 1024)
                psl = slice(g * 128, (g + 1) * 128)
                _gate_group(
                    nc, psum, gg, W, 128, c5[:, sl], h5[:, sl],
                    x15[:, L4_OFF + psl.start : L4_OFF + psl.stop],
                    hs4[:, psl], fc4[:, psl],
                )

            def l4top_half(g):
                sl = slice(g * 256, (g + 1) * 256)
                _level_top_small(
                    nc, psum, gp, W, 256,
                    x15[:, L4_OFF + sl.start : L4_OFF + sl.stop],
                    hs4[:, sl], fc4[:, sl], c4[:, sl], h4[:, sl],
                )

            # ---- Phase 1: leaves fused with L5 forget-gates/child-sums.
            # The L5 gate stage for chunk ch-1 is issued after the leaves of
            # chunk ch (one full chunk of slack), and the L5-top / L4 stages
            # are interleaved with >=1 chunk of slack, so every inserted
            # chain's inputs are long ready and the ACT stream never stalls.
            leaf_states = {}

            def leaf_chunk(ch):
                # Same per-group op order as _level_top's leaf path, except
                # tanh(c) runs once per chunk as a single 4096-wide ACT op
                # (halves that op's per-op overhead).
                lc = lf_p.tile([128, 4096], F16, tag="lc", name=f"lc{ch}")
                lh = lf_p.tile([128, 4096], F16, tag="lh", name=f"lh{ch}")
                leaf_states[ch] = (lc, lh)
                so4 = gp.tile([128, 4096], F16, tag="so4", name=f"so4_{ch}")
                for g in range(2):
                    if (ch, g) not in x6_tiles:
                        t = x6_p.tile(
                            [128, 2048], F16, tag="x6", name=f"x6_{ch}_{g}"
                        )
                        base = LEAF_OFF + ch * 4096 + g * 2048
                        nc.sync.dma_start(t[:], xs[:, base : base + 2048])
                        x6_tiles[(ch, g)] = t
                    sl = slice(g * 2048, (g + 1) * 2048)
                    x6t = x6_tiles[(ch, g)]

                    def gate_psum(gate, name):
                        p = psum.tile([128, 2048], F32, tag="pg", name=name)
                        w = W["wioux"][:, gate * 128 : (gate + 1) * 128]
                        for s in range(0, 2048, 512):
                            nc.tensor.matmul(
                                p[:, s : s + 512], w, x6t[:, s : s + 512],
                                start=True, stop=True,
                            )
                        return p

                    pi = gate_psum(0, f"pi6_{ch}_{g}")
                    pu = gate_psum(2, f"pu6_{ch}_{g}")
                    si = gp.tile([128, 2048], F16, tag="si", name=f"si6_{ch}_{g}")
                    nc.scalar.activation(si[:], pi[:], SIG, bias=W["biou"][:, 0:1])
                    tu = gp.tile([128, 2048], F16, tag="tu", name=f"tu6_{ch}_{g}")
                    nc.scalar.activation(tu[:], pu[:], TANH, bias=W["biou"][:, 2:3])
                    po = gate_psum(1, f"po6_{ch}_{g}")
                    nc.vector.tensor_mul(lc[:, sl], si[:], tu[:])
                    nc.scalar.activation(
                        so4[:, sl], po[:], SIG, bias=W["biou"][:, 1:2]
                    )
                tct4 = gp.tile([128, 4096], F16, tag="tct4", name=f"tct4_{ch}")
                nc.scalar.activation(tct4[:], lc[:], TANH)
                for g in range(2):
                    sl = slice(g * 2048, (g + 1) * 2048)
                    nc.vector.tensor_mul(lh[:, sl], so4[:, sl], tct4[:, sl])

            def l5_gates(ch):
                lc, lh = leaf_states.pop(ch)
                for g in range(2):
                    sl = slice(g * 2048, (g + 1) * 2048)
                    psl = slice(ch * 512 + g * 256, ch * 512 + (g + 1) * 256)
                    xp = x15[:, L5_OFF + psl.start : L5_OFF + psl.stop]
                    _gate_group(
                        nc, psum, gg, W, 256, lc[:, sl], lh[:, sl], xp,
                        hs5[:, psl], fc5[:, psl],
                    )

            for ch in range(N_CHUNKS):
                leaf_chunk(ch)
                if ch == 5:
                    l5top_p2(0)
                elif ch == 6:
                    l5top_p2(1)
                elif ch == 7:
                    l5top_p2(2)
                if ch >= 1:
                    l5_gates(ch - 1)
                if ch == 4:
                    l5top_p1(0)
                elif ch == 5:
                    l5top_p1(1)
                elif ch == 6:
                    l5top_p1(2)
                    l4gate(0)
                elif ch == 7:
                    l4gate(1)

            l5_gates(7)
            l4gate(2)
            l4top_half(0)
            l5top_p1(3)
            l5top_p2(3)
            l4gate(3)
            l4top_half(1)

            # ---- L3 (64 nodes) ----
            _gate_group(
                nc, psum, gg, W, 64, c4[:], h4[:],
                x15[:, LVL_OFF[2] : LVL_OFF[2] + 64], hs3[:], fc3[:], small=True,
            )
            _level_top_small(
                nc, psum, gp, W, 64,
                x15[:, LVL_OFF[2] : LVL_OFF[2] + 64], hs3[:], fc3[:], c3[:], h3[:],
            )

            # ---- L2 (8 nodes) ----
            _gate_group(
                nc, psum, gg, W, 8, c3[:], h3[:],
                x15[:, LVL_OFF[1] : LVL_OFF[1] + 8], hs2[:], fc2[:], small=True,
            )
            _level_top_small(
                nc, psum, gp, W, 8,
                x15[:, LVL_OFF[1] : LVL_OFF[1] + 8], hs2[:], fc2[:], c2[:], h2[:],
            )

            # ---- L1 (subtree root) -> fp32 out ----
            _gate_group(
                nc, psum, gg, W, 1, c2[:], h2[:],
                x15[:, 0:1], hs1[:], fc1[:], small=True,
            )
            _level_top_small(
                nc, psum, gp, W, 1,
                x15[:, 0:1], hs1[:], fc1[:],
                out_ch[:, 0:1], out_ch[:, 1:2], out_dt=F32,
            )
            nc.sync.dma_start(out_d, out_ch[:])

    nc.compile()
    return nc


_NC_CACHE = None


def _get_nc():
    global _NC_CACHE
    if _NC_CACHE is None:
        _NC_CACHE = _build_subtree_kernel()
    return _NC_CACHE


def _sigmoid(x):
    return 1.0 / (1.0 + np.exp(-x))


def kernel(
    x, W_ioux, b_ioux, W_iouh, b_iouh, W_fx, b_fx, W_fh, b_fh, branch, depth
):
    global LAST_RESULTS
    assert int(branch) == BRANCH and int(depth) == DEPTH

    x = np.asarray(x, np.float32)
    W_ioux = np.asarray(W_ioux, np.float32)
    b_ioux = np.asarray(b_ioux, np.float32)
    W_iouh = np.asarray(W_iouh, np.float32)
    b_iouh = np.asarray(b_iouh, np.float32)
    W_fx = np.asarray(W_fx, np.float32)
    b_fx = np.asarray(b_fx, np.float32)
    W_fh = np.asarray(W_fh, np.float32)
    b_fh = np.asarray(b_fh, np.float32)

    wioux = np.ascontiguousarray(W_ioux.T.astype(np.float16))
    wiouh = np.ascontiguousarray(W_iouh.T.astype(np.float16))
    wfx = np.ascontiguousarray(W_fx.T.astype(np.float16))
    wfh = np.ascontiguousarray(W_fh.T.astype(np.float16))
    biou_full = b_ioux + b_iouh
    biou = np.ascontiguousarray(biou_full.reshape(3, 128).T.astype(np.float32))
    biourow = np.ascontiguousarray(biou_full.reshape(1, 384).astype(np.float16))
    bf = np.ascontiguousarray((b_fx + b_fh).reshape(128, 1).astype(np.float32))

    off = lambda l: (BRANCH**l - 1) // (BRANCH - 1)
    in_maps = []
    for c in range(BRANCH):
        parts = []
        for l in range(1, DEPTH + 1):
            sz = BRANCH ** (l - 1)
            parts.append(x[off(l) + c * sz : off(l) + (c + 1) * sz])
        xs_c = np.ascontiguousarray(
            np.concatenate(parts, axis=0).T.astype(np.float16)
        )
        in_maps.append(
            {
                "xs": xs_c,
                "wioux": wioux,
                "wiouh": wiouh,
                "wfx": wfx,
                "wfh": wfh,
                "biou": biou,
                "biourow": biourow,
                "bf": bf,
            }
        )

    nc = _get_nc()
    trace = os.environ.get("TREELSTM_TRACE") == "1"
    res = run_bass_kernel_spmd(nc, in_maps, core_ids=list(range(8)), trace=trace)
    LAST_RESULTS = res

    c_ch = np.stack([res.results[c]["out"][:, 0] for c in range(8)])  # [8,128]
    h_ch = np.stack([res.results[c]["out"][:, 1] for c in range(8)])  # [8,128]

    # Root node on host (fp32), matching reference node_forward.
    x0 = x[0:1]  # [1,128]
    h_sum = h_ch.sum(axis=0, keepdims=True)  # [1,128]
    f = _sigmoid(h_ch @ W_fh.T + b_fh + (x0 @ W_fx.T + b_fx))  # [8,128]
    fc_sum = (f * c_ch).sum(axis=0, keepdims=True)  # [1,128]
    iou = x0 @ W_ioux.T + b_ioux + h_sum @ W_iouh.T + b_iouh  # [1,384]
    i, o, u = iou[:, 0:128], iou[:, 128:256], iou[:, 256:384]
    c_root = _sigmoid(i) * np.tanh(u) + fc_sum
    h_root = _sigmoid(o) * np.tanh(c_root)
    return (c_root.astype(np.float32), h_root.astype(np.float32))

